# revision 14
# baseline (speedup 1.0000x reference)
"""Bahdanau attention Trainium2 kernel.

  qk   = tanh(k @ W1 + q @ W2)          (B, Sk, dk)
  z    = qk @ Wf + bf                   (B, Sk, 1)
  p    = softmax(z, axis=1)             (B, Sk, 1)
  attn = sum(p * v, axis=1)             (B, dv)
  returns (attn, p)

Sharding: pure data-parallel over batch. 8 cores x 4 batches each, no
collectives. Per-core inputs are the 4-batch shards of k/v (bf16) plus
small replicated weights.

Per-batch on-chip pipeline (phases keep the PE stream dense):
  A. HWDGE DMA-transpose loads kT[d, s] (bf16) straight from DRAM; then
     16 matmuls (lhsT=W1[d,e], rhs=kT[d,s-chunk]) -> y[e, s] PSUM, and 8
     ACT passes t = tanh(y + c_b) (c = q@W2 per-partition bias) -> bf16.
  B. 64 back-to-back matmuls (lhsT=t[:,128-slice], rhs=Wf[e,1]) -> z
     columns accumulate into one PSUM tile zb[128,64] (s = col*128+part).
  C. ACT exp over zb with accum_out -> p_un[128,64] f32 + rowsum[128,1];
     matmul(lhsT=ones, rhs=rowsum) broadcasts Z; DVE reciprocal ->
     rz[128,1]; DVE makes pn_f32 (output path) and pn_bf (wsum path).
  D. PE transpose of pn_f32 via identity -> [64,128]; DVE copy; SWDGE DMA
     to DRAM (s-contiguous rows).
  E. 64 back-to-back matmuls (lhsT=pn_bf[:,c], rhs=v_tile[s=128,dv] bf16)
     accumulating -> attn[1, dv]; copy; DMA out.

bf16 is used for k/W1/Wf/tanh-output/v/p-weights; softmax statistics and
PSUM accumulation stay f32.
"""

import os
import sys

for _p in ("/opt/trn_rl_repo",):
    if _p not in sys.path and os.path.isdir(_p):
        sys.path.append(_p)

import ml_dtypes
import numpy as np

B, SK, D = 32, 8192, 128
NCORES = 8
BPC = B // NCORES          # batches per core
CHUNK = 1024               # tanh chunk (columns of kT); 2 matmuls of 512
NCHUNK = SK // CHUNK       # 8
NSLICE = SK // 128         # 64 z-columns / wsum slices per batch
KSPLIT = 2                 # k DMAs per batch (1 per HWDGE ring)
VSPLIT = 2                 # v DMAs per batch
_BF16 = ml_dtypes.bfloat16

_graph_cache = {}
LAST_RESULT = None


def _patch_walrus_flags():
    """Optionally flip --enable-ldw-opt for the walrus invocation (FWL)."""
    if not int(os.environ.get("KERNEL_LDW_OPT", "0")):
        return
    import subprocess as sp
    if getattr(sp, "_kernel_ldw_patched", False):
        return
    orig = sp.check_call

    def patched(argv, *a, **k):
        if isinstance(argv, list) and argv and "walrus_driver" in str(argv[0]):
            argv = [
                x.replace("--enable-ldw-opt=false", "--enable-ldw-opt=true")
                if isinstance(x, str) else x
                for x in argv
            ]
        return orig(argv, *a, **k)

    sp.check_call = patched
    sp._kernel_ldw_patched = True


def _split_multi_sync(nc):
    """This container's walrus lowers each TPB instruction with a single
    sync-wait/update slot (NEURON_ISA_TPB_EVENTS) and rejects instructions
    carrying more ("Too many sync wait commands").  Tile emits coalesced
    multi-wait instructions, so split the extras onto same-engine NOPs.
    Semaphores are monotonic within a kernel, so waiting sequentially is
    equivalent to waiting jointly."""
    from concourse import mybir

    fn = nc.m.functions[0]
    for blk in fn.blocks:
        new = []
        changed = False
        for inst in blk.instructions:
            si = getattr(inst, "sync_info", None)
            waits = list(si.on_wait) if si is not None and si.on_wait else []
            upds = list(si.on_update) if si is not None and si.on_update else []
            if len(waits) > 1:
                for j, w in enumerate(waits[:-1]):
                    new.append(mybir.InstNoOp(
                        name=f"{inst.name}-sw{j}",
                        engine=inst.engine,
                        sync_info=mybir.SyncInfo(on_wait=[w], on_update=[]),
                        bass_nofuse=True,
                    ))
                inst.sync_info = mybir.SyncInfo(on_wait=[waits[-1]], on_update=upds)
                changed = True
            new.append(inst)
            if len(upds) > 1:
                inst.sync_info = mybir.SyncInfo(
                    on_wait=list(inst.sync_info.on_wait), on_update=[upds[0]]
                )
                for j, u in enumerate(upds[1:]):
                    new.append(mybir.InstNoOp(
                        name=f"{inst.name}-su{j}",
                        engine=inst.engine,
                        sync_info=mybir.SyncInfo(on_wait=[], on_update=[u]),
                        bass_nofuse=True,
                    ))
                changed = True
        if changed:
            blk.instructions = new


def _build_graph():
    import concourse.bass as bass
    import concourse.tile as tile
    from concourse import mybir

    f32 = mybir.dt.float32
    bf16 = mybir.dt.bfloat16

    nc = bass.Bass()

    k_in = nc.declare_dram_parameter("k", [BPC, D, SK], bf16, isOutput=False)
    v_in = nc.declare_dram_parameter("v", [BPC, SK, D], bf16, isOutput=False)
    w1_in = nc.declare_dram_parameter("w1", [D, D], bf16, isOutput=False)
    wf_in = nc.declare_dram_parameter("wf", [D, 1], bf16, isOutput=False)
    ct_in = nc.declare_dram_parameter("ct", [D, BPC], f32, isOutput=False)
    id_in = nc.declare_dram_parameter("ident", [D, D], f32, isOutput=False)
    attn_out = nc.declare_dram_parameter("attn", [BPC, D], f32, isOutput=True)
    asm_out = nc.declare_dram_parameter("attn_sm", [BPC, SK], f32, isOutput=True)

    ks_rows = SK // KSPLIT                 # k rows per transpose-DMA
    vs_rows = SK // VSPLIT                 # v rows per DMA
    ZPC = CHUNK // 128                     # z slices per chunk (8)

    with tile.TileContext(nc) as tc:
        with (
            tc.tile_pool(name="consts", bufs=1) as consts,
            tc.tile_pool(name="kt", bufs=3 * KSPLIT) as kt_pool,
            tc.tile_pool(name="vt", bufs=2 * VSPLIT) as vt_pool,
            tc.tile_pool(name="t", bufs=NCHUNK + 3) as t_pool,
            tc.tile_pool(name="sm", bufs=3) as sm_pool,
            tc.tile_pool(name="yps", bufs=2, space="PSUM") as yps_pool,
            tc.tile_pool(name="zps", bufs=2, space="PSUM") as zps_pool,
            tc.tile_pool(name="sps", bufs=2, space="PSUM") as sps_pool,
        ):
            w1_sb = consts.tile([D, D], bf16)
            nc.sync.dma_start(out=w1_sb[:], in_=w1_in[:])
            wf_sb = consts.tile([D, 1], bf16)
            nc.sync.dma_start(out=wf_sb[:], in_=wf_in[:])
            ct_sb = consts.tile([D, BPC], f32)
            nc.sync.dma_start(out=ct_sb[:], in_=ct_in[:])
            id_sb = consts.tile([D, D], f32)
            nc.sync.dma_start(out=id_sb[:], in_=id_in[:])
            ones_sb = consts.tile([D, D], f32)
            nc.vector.memset(ones_sb[:], 1.0)

            # Software pipeline over batches: during step b the PE runs
            # phase A of batch b (high-duty N=512 matmuls, keeps HAM warm)
            # with batch b's z-slices one chunk behind tanh, interleaved
            # with the weighted sum of batch b-1.  DMAs prefetch one batch
            # ahead on separate rings (sync: k transposes, gpsimd: v).
            kts = {}
            vts = {}
            zbs = {}
            pns = {}

            def emit_kt(b):
                tiles = []
                for g in range(KSPLIT):
                    kt = kt_pool.tile([D, ks_rows], bf16, tag="kt")
                    eng = nc.sync if g % 2 == 0 else nc.scalar
                    eng.dma_start(
                        out=kt[:],
                        in_=k_in[b, :, g * ks_rows:(g + 1) * ks_rows],
                    )
                    tiles.append(kt)
                kts[b] = tiles

            def emit_vt(b):
                tiles = []
                for g in range(VSPLIT):
                    vt = vt_pool.tile([128, vs_rows // 128, D], bf16, tag="vt")
                    nc.gpsimd.dma_start(
                        out=vt[:],
                        in_=v_in[b, g * vs_rows:(g + 1) * vs_rows, :].rearrange(
                            "(c s) d -> s c d", s=128
                        ),
                    )
                    tiles.append(vt)
                vts[b] = tiles

            def emit_z_slices(b, ch):
                zb = zbs[b]
                t = zbs[(b, ch)]
                for u in range(ZPC):
                    sl = ch * ZPC + u
                    nc.tensor.matmul(
                        zb[:, sl:sl + 1], t[:, u * 128:(u + 1) * 128], wf_sb[:],
                        start=True, stop=True, skip_group_check=True,
                    )

            def emit_softmax(b):
                zb = zbs[b]
                p_un = sm_pool.tile([D, NSLICE], f32, tag="p_un")
                rowsum = sm_pool.tile([D, 1], f32, tag="rowsum")
                nc.scalar.activation(
                    p_un[:], zb[:], mybir.ActivationFunctionType.Exp,
                    accum_out=rowsum[:],
                )
                zt = sps_pool.tile([D, 1], f32, tag="sps")
                nc.tensor.matmul(zt[:], ones_sb[:], rowsum[:], start=True,
                                 stop=True, skip_group_check=True)
                rz = sm_pool.tile([D, 1], f32, tag="rz")
                nc.vector.reciprocal(rz[:], zt[:])
                pn_f32 = sm_pool.tile([D, NSLICE], f32, tag="pn_f32")
                nc.vector.tensor_scalar_mul(pn_f32[:], p_un[:], rz[:])
                pn_bf = sm_pool.tile([D, NSLICE], bf16, tag="pn_bf")
                nc.vector.tensor_scalar_mul(pn_bf[:], p_un[:], rz[:])
                pns[b] = pn_bf
                pns[(b, "f32")] = pn_f32

            def emit_asm_out(b):
                # attn_sm out via PE transpose to s-major rows
                pn_f32 = pns.pop((b, "f32"))
                pt_ps = sps_pool.tile([NSLICE, D], f32, tag="sps")
                nc.tensor.matmul(
                    pt_ps[:], pn_f32[:], id_sb[:],
                    start=True, stop=True, is_transpose=True,
                    skip_group_check=True,
                )
                pt_sb = sm_pool.tile([NSLICE, D], f32, tag="pt_sb")
                nc.vector.tensor_copy(pt_sb[:], pt_ps[:])
                nc.gpsimd.dma_start(
                    out=asm_out[b].rearrange("(a d) -> a d", d=D),
                    in_=pt_sb[:],
                )

            emit_kt(0)
            if BPC > 1:
                emit_kt(1)
            emit_vt(0)

            for step in range(BPC + 1):
                a = step          # phase-A batch
                w = step - 1      # softmax + wsum batch
                if a < BPC:
                    if a + 2 < BPC:
                        emit_kt(a + 2)
                    if a + 1 < BPC:
                        emit_vt(a + 1)
                    zbs[a] = zps_pool.tile([D, NSLICE], f32, tag="zb", name="zb")
                if w >= 0:
                    emit_softmax(w)
                    attn_ps = sps_pool.tile([1, D], f32, tag="sps")
                    pn_bf = pns.pop(w)

                for ch in range(NCHUNK):
                    if w >= 0:
                        vt = vts[w][(ch * CHUNK) // vs_rows]
                        for u in range(ZPC):
                            c_idx = ch * ZPC + u
                            voff = (ch * CHUNK) % vs_rows // 128 + u
                            nc.tensor.matmul(
                                attn_ps[:],
                                pn_bf[:, c_idx:c_idx + 1],
                                vt[:, voff, :],
                                start=(c_idx == 0),
                                stop=(c_idx == NSLICE - 1),
                                skip_group_check=True,
                            )
                    if a < BPC and ch > 0:
                        emit_z_slices(a, ch - 1)
                    if a < BPC:
                        kt = kts[a][(ch * CHUNK) // ks_rows]
                        coff = (ch * CHUNK) % ks_rows
                        y = yps_pool.tile([D, CHUNK], f32, tag="y")
                        nc.tensor.matmul(
                            y[:, 0:512], w1_sb[:], kt[:, coff:coff + 512],
                            start=True, stop=True, skip_group_check=True,
                        )
                        nc.tensor.matmul(
                            y[:, 512:CHUNK], w1_sb[:],
                            kt[:, coff + 512:coff + CHUNK],
                            start=True, stop=True, skip_group_check=True,
                        )
                        t = t_pool.tile([D, CHUNK], bf16, tag="t")
                        nc.scalar.activation(
                            t[:], y[:], mybir.ActivationFunctionType.Tanh,
                            bias=ct_sb[:, a:a + 1], scale=1.0,
                        )
                        zbs[(a, ch)] = t

                if w >= 0:
                    attn_sb = sm_pool.tile([1, D], f32, tag="attn_sb")
                    nc.vector.tensor_copy(attn_sb[:], attn_ps[:])
                    nc.gpsimd.dma_start(out=attn_out[w:w + 1, :], in_=attn_sb[:])
                    emit_asm_out(w)
                    vts.pop(w, None)
                    zbs.pop(w, None)
                    for ch in range(NCHUNK):
                        zbs.pop((w, ch), None)
                if a < BPC:
                    emit_z_slices(a, NCHUNK - 1)
                    kts.pop(a, None)

    _split_multi_sync(nc)
    return nc


def kernel(q, k, v, W1, W2, Wf, bf):
    global LAST_RESULT
    _patch_walrus_flags()
    from concourse.bass_utils import run_bass_kernel_spmd

    q = np.asarray(q, dtype=np.float32)
    k = np.asarray(k, dtype=np.float32)
    v = np.asarray(v, dtype=np.float32)
    W1 = np.asarray(W1, dtype=np.float32)
    W2 = np.asarray(W2, dtype=np.float32)
    Wf = np.asarray(Wf, dtype=np.float32)
    bf = np.asarray(bf, dtype=np.float32)

    if "nc" not in _graph_cache:
        _graph_cache["nc"] = _build_graph()
    nc = _graph_cache["nc"]

    # q @ W2 is (B,128)@(128,128): do it on host, it is the per-batch tanh
    # bias.  bf shifts every logit of a batch equally -> softmax-invariant,
    # so it drops out of both outputs.
    c = (q[:, 0, :].astype(np.float64) @ W2.astype(np.float64)).astype(np.float32)

    k_bf = np.ascontiguousarray(k.astype(_BF16).transpose(0, 2, 1))
    v_bf = v.astype(_BF16)
    w1_bf = W1.astype(_BF16)
    wf_bf = Wf.astype(_BF16)
    ident = np.eye(D, dtype=np.float32)

    in_maps = []
    for i in range(NCORES):
        sl = slice(i * BPC, (i + 1) * BPC)
        in_maps.append({
            "k": k_bf[sl],
            "v": v_bf[sl],
            "w1": w1_bf,
            "wf": wf_bf,
            "ct": np.ascontiguousarray(c[sl].T),
            "ident": ident,
        })

    res = run_bass_kernel_spmd(
        nc, in_maps, list(range(NCORES)),
        trace=bool(int(os.environ.get("KERNEL_TRACE", "0"))),
    )
    LAST_RESULT = res

    attn = np.concatenate([r["attn"] for r in res.results], axis=0)
    attn_sm = np.concatenate([r["attn_sm"] for r in res.results], axis=0)
    return attn.astype(np.float32), attn_sm.astype(np.float32).reshape(B, SK, 1)


# revision 15
# speedup vs baseline: 1.0929x; 1.0929x over previous
"""Bahdanau attention Trainium2 kernel.

  qk   = tanh(k @ W1 + q @ W2)          (B, Sk, dk)
  z    = qk @ Wf + bf                   (B, Sk, 1)
  p    = softmax(z, axis=1)             (B, Sk, 1)
  attn = sum(p * v, axis=1)             (B, dv)
  returns (attn, p)

Sharding: pure data-parallel over batch. 8 cores x 4 batches each, no
collectives. Per-core inputs are the 4-batch shards of k/v (bf16) plus
small replicated weights.

Per-batch on-chip pipeline (phases keep the PE stream dense):
  A. HWDGE DMA-transpose loads kT[d, s] (bf16) straight from DRAM; then
     16 matmuls (lhsT=W1[d,e], rhs=kT[d,s-chunk]) -> y[e, s] PSUM, and 8
     ACT passes t = tanh(y + c_b) (c = q@W2 per-partition bias) -> bf16.
  B. 64 back-to-back matmuls (lhsT=t[:,128-slice], rhs=Wf[e,1]) -> z
     columns accumulate into one PSUM tile zb[128,64] (s = col*128+part).
  C. ACT exp over zb with accum_out -> p_un[128,64] f32 + rowsum[128,1];
     matmul(lhsT=ones, rhs=rowsum) broadcasts Z; DVE reciprocal ->
     rz[128,1]; DVE makes pn_f32 (output path) and pn_bf (wsum path).
  D. PE transpose of pn_f32 via identity -> [64,128]; DVE copy; SWDGE DMA
     to DRAM (s-contiguous rows).
  E. 64 back-to-back matmuls (lhsT=pn_bf[:,c], rhs=v_tile[s=128,dv] bf16)
     accumulating -> attn[1, dv]; copy; DMA out.

bf16 is used for k/W1/Wf/tanh-output/v/p-weights; softmax statistics and
PSUM accumulation stay f32.
"""

import os
import sys

for _p in ("/opt/trn_rl_repo",):
    if _p not in sys.path and os.path.isdir(_p):
        sys.path.append(_p)

import ml_dtypes
import numpy as np

B, SK, D = 32, 8192, 128
NCORES = 8
BPC = B // NCORES          # batches per core
CHUNK = 1024               # tanh chunk (columns of kT); 2 matmuls of 512
NCHUNK = SK // CHUNK       # 8
NSLICE = SK // 128         # 64 z-columns / wsum slices per batch
KSPLIT = 4                 # k DMAs per batch
VSPLIT = 4                 # v DMAs per batch
_BF16 = ml_dtypes.bfloat16

_graph_cache = {}
LAST_RESULT = None


def _patch_walrus_flags():
    """Optionally flip --enable-ldw-opt for the walrus invocation (FWL)."""
    if not int(os.environ.get("KERNEL_LDW_OPT", "0")):
        return
    import subprocess as sp
    if getattr(sp, "_kernel_ldw_patched", False):
        return
    orig = sp.check_call

    def patched(argv, *a, **k):
        if isinstance(argv, list) and argv and "walrus_driver" in str(argv[0]):
            argv = [
                x.replace("--enable-ldw-opt=false", "--enable-ldw-opt=true")
                if isinstance(x, str) else x
                for x in argv
            ]
        return orig(argv, *a, **k)

    sp.check_call = patched
    sp._kernel_ldw_patched = True


def _split_multi_sync(nc):
    """This container's walrus lowers each TPB instruction with a single
    sync-wait/update slot (NEURON_ISA_TPB_EVENTS) and rejects instructions
    carrying more ("Too many sync wait commands").  Tile emits coalesced
    multi-wait instructions, so split the extras onto same-engine NOPs.
    Semaphores are monotonic within a kernel, so waiting sequentially is
    equivalent to waiting jointly."""
    from concourse import mybir

    fn = nc.m.functions[0]
    for blk in fn.blocks:
        new = []
        changed = False
        for inst in blk.instructions:
            si = getattr(inst, "sync_info", None)
            waits = list(si.on_wait) if si is not None and si.on_wait else []
            upds = list(si.on_update) if si is not None and si.on_update else []
            if len(waits) > 1:
                for j, w in enumerate(waits[:-1]):
                    new.append(mybir.InstNoOp(
                        name=f"{inst.name}-sw{j}",
                        engine=inst.engine,
                        sync_info=mybir.SyncInfo(on_wait=[w], on_update=[]),
                        bass_nofuse=True,
                    ))
                inst.sync_info = mybir.SyncInfo(on_wait=[waits[-1]], on_update=upds)
                changed = True
            new.append(inst)
            if len(upds) > 1:
                inst.sync_info = mybir.SyncInfo(
                    on_wait=list(inst.sync_info.on_wait), on_update=[upds[0]]
                )
                for j, u in enumerate(upds[1:]):
                    new.append(mybir.InstNoOp(
                        name=f"{inst.name}-su{j}",
                        engine=inst.engine,
                        sync_info=mybir.SyncInfo(on_wait=[], on_update=[u]),
                        bass_nofuse=True,
                    ))
                changed = True
        if changed:
            blk.instructions = new


def _build_graph():
    import concourse.bass as bass
    import concourse.tile as tile
    from concourse import mybir

    f32 = mybir.dt.float32
    bf16 = mybir.dt.bfloat16

    nc = bass.Bass()

    k_in = nc.declare_dram_parameter("k", [BPC, D, SK], bf16, isOutput=False)
    v_in = nc.declare_dram_parameter("v", [BPC, SK, D], bf16, isOutput=False)
    w1_in = nc.declare_dram_parameter("w1", [D, D], bf16, isOutput=False)
    wf_in = nc.declare_dram_parameter("wf", [D, 1], bf16, isOutput=False)
    ct_in = nc.declare_dram_parameter("ct", [D, BPC], f32, isOutput=False)
    id_in = nc.declare_dram_parameter("ident", [D, D], f32, isOutput=False)
    attn_out = nc.declare_dram_parameter("attn", [BPC, D], f32, isOutput=True)
    asm_out = nc.declare_dram_parameter("attn_sm", [BPC, SK], f32, isOutput=True)

    ks_rows = SK // KSPLIT                 # k rows per transpose-DMA
    vs_rows = SK // VSPLIT                 # v rows per DMA
    ZPC = CHUNK // 128                     # z slices per chunk (8)

    with tile.TileContext(nc) as tc:
        with (
            tc.tile_pool(name="consts", bufs=1) as consts,
            tc.tile_pool(name="kt", bufs=3 * KSPLIT) as kt_pool,
            tc.tile_pool(name="vt", bufs=2 * VSPLIT) as vt_pool,
            tc.tile_pool(name="t", bufs=NCHUNK + 3) as t_pool,
            tc.tile_pool(name="sm", bufs=3) as sm_pool,
            tc.tile_pool(name="yps", bufs=2, space="PSUM") as yps_pool,
            tc.tile_pool(name="zps", bufs=2, space="PSUM") as zps_pool,
            tc.tile_pool(name="sps", bufs=2, space="PSUM") as sps_pool,
        ):
            w1_sb = consts.tile([D, D], bf16)
            nc.gpsimd.dma_start(out=w1_sb[:], in_=w1_in[:])
            wf_sb = consts.tile([D, 1], bf16)
            nc.gpsimd.dma_start(out=wf_sb[:], in_=wf_in[:])
            ct_sb = consts.tile([D, BPC], f32)
            nc.gpsimd.dma_start(out=ct_sb[:], in_=ct_in[:])
            id_sb = consts.tile([D, D], f32)
            nc.gpsimd.dma_start(out=id_sb[:], in_=id_in[:])
            ones_sb = consts.tile([D, D], f32)
            nc.vector.memset(ones_sb[:], 1.0)

            # Software pipeline over batches: during step b the PE runs
            # phase A of batch b (high-duty N=512 matmuls, keeps HAM warm)
            # with batch b's z-slices one chunk behind tanh, interleaved
            # with the weighted sum of batch b-1.  DMAs prefetch one batch
            # ahead on separate rings (sync: k transposes, gpsimd: v).
            kts = {}
            vts = {}
            zbs = {}
            pns = {}

            def emit_kt(b):
                tiles = []
                for g in range(KSPLIT):
                    kt = kt_pool.tile([D, ks_rows], bf16, tag="kt")
                    nc.sync.dma_start(
                        out=kt[:],
                        in_=k_in[b, :, g * ks_rows:(g + 1) * ks_rows],
                    )
                    tiles.append(kt)
                kts[b] = tiles

            def emit_vt(b):
                tiles = []
                for g in range(VSPLIT):
                    vt = vt_pool.tile([128, vs_rows // 128, D], bf16, tag="vt")
                    nc.gpsimd.dma_start(
                        out=vt[:],
                        in_=v_in[b, g * vs_rows:(g + 1) * vs_rows, :].rearrange(
                            "(c s) d -> s c d", s=128
                        ),
                    )
                    tiles.append(vt)
                vts[b] = tiles

            def emit_z_slices(b, ch):
                zb = zbs[b]
                t = zbs[(b, ch)]
                for u in range(ZPC):
                    sl = ch * ZPC + u
                    nc.tensor.matmul(
                        zb[:, sl:sl + 1], t[:, u * 128:(u + 1) * 128], wf_sb[:],
                        start=True, stop=True, skip_group_check=True,
                    )

            def emit_softmax(b):
                zb = zbs[b]
                p_un = sm_pool.tile([D, NSLICE], f32, tag="p_un")
                rowsum = sm_pool.tile([D, 1], f32, tag="rowsum")
                nc.scalar.activation(
                    p_un[:], zb[:], mybir.ActivationFunctionType.Exp,
                    accum_out=rowsum[:],
                )
                zt = sps_pool.tile([D, 1], f32, tag="sps")
                nc.tensor.matmul(zt[:], ones_sb[:], rowsum[:], start=True,
                                 stop=True, skip_group_check=True)
                rz = sm_pool.tile([D, 1], f32, tag="rz")
                nc.vector.reciprocal(rz[:], zt[:])
                pn_f32 = sm_pool.tile([D, NSLICE], f32, tag="pn_f32")
                nc.vector.tensor_scalar_mul(pn_f32[:], p_un[:], rz[:])
                pn_bf = sm_pool.tile([D, NSLICE], bf16, tag="pn_bf")
                nc.vector.tensor_scalar_mul(pn_bf[:], p_un[:], rz[:])
                pns[b] = pn_bf
                pns[(b, "f32")] = pn_f32

            def emit_asm_out(b):
                # attn_sm out via PE transpose to s-major rows
                pn_f32 = pns.pop((b, "f32"))
                pt_ps = sps_pool.tile([NSLICE, D], f32, tag="sps")
                nc.tensor.matmul(
                    pt_ps[:], pn_f32[:], id_sb[:],
                    start=True, stop=True, is_transpose=True,
                    skip_group_check=True,
                )
                pt_sb = sm_pool.tile([NSLICE, D], f32, tag="pt_sb")
                nc.vector.tensor_copy(pt_sb[:], pt_ps[:])
                nc.gpsimd.dma_start(
                    out=asm_out[b].rearrange("(a d) -> a d", d=D),
                    in_=pt_sb[:],
                )

            emit_kt(0)
            if BPC > 1:
                emit_kt(1)
            emit_vt(0)

            for step in range(BPC + 1):
                a = step          # phase-A batch
                w = step - 1      # softmax + wsum batch
                if a < BPC:
                    if a + 2 < BPC:
                        emit_kt(a + 2)
                    if a + 1 < BPC:
                        emit_vt(a + 1)
                    zbs[a] = zps_pool.tile([D, NSLICE], f32, tag="zb", name="zb")
                if w >= 0:
                    emit_softmax(w)
                    attn_ps = sps_pool.tile([1, D], f32, tag="sps")
                    pn_bf = pns.pop(w)

                for ch in range(NCHUNK):
                    if a < BPC:
                        kt = kts[a][(ch * CHUNK) // ks_rows]
                        coff = (ch * CHUNK) % ks_rows
                        y = yps_pool.tile([D, CHUNK], f32, tag="y")
                        nc.tensor.matmul(
                            y[:, 0:512], w1_sb[:], kt[:, coff:coff + 512],
                            start=True, stop=True, skip_group_check=True,
                        )
                        nc.tensor.matmul(
                            y[:, 512:CHUNK], w1_sb[:],
                            kt[:, coff + 512:coff + CHUNK],
                            start=True, stop=True, skip_group_check=True,
                        )
                        t = t_pool.tile([D, CHUNK], bf16, tag="t")
                        nc.scalar.activation(
                            t[:], y[:], mybir.ActivationFunctionType.Tanh,
                            bias=ct_sb[:, a:a + 1], scale=1.0,
                        )
                        zbs[(a, ch)] = t
                        if ch > 0:
                            emit_z_slices(a, ch - 1)
                    if w >= 0:
                        vt = vts[w][(ch * CHUNK) // vs_rows]
                        for u in range(ZPC):
                            c_idx = ch * ZPC + u
                            voff = (ch * CHUNK) % vs_rows // 128 + u
                            nc.tensor.matmul(
                                attn_ps[:],
                                pn_bf[:, c_idx:c_idx + 1],
                                vt[:, voff, :],
                                start=(c_idx == 0),
                                stop=(c_idx == NSLICE - 1),
                                skip_group_check=True,
                            )

                if w >= 0:
                    attn_sb = sm_pool.tile([1, D], f32, tag="attn_sb")
                    nc.vector.tensor_copy(attn_sb[:], attn_ps[:])
                    nc.gpsimd.dma_start(out=attn_out[w:w + 1, :], in_=attn_sb[:])
                    emit_asm_out(w)
                    vts.pop(w, None)
                    zbs.pop(w, None)
                    for ch in range(NCHUNK):
                        zbs.pop((w, ch), None)
                if a < BPC:
                    emit_z_slices(a, NCHUNK - 1)
                    kts.pop(a, None)

    _split_multi_sync(nc)
    return nc


def kernel(q, k, v, W1, W2, Wf, bf):
    global LAST_RESULT
    _patch_walrus_flags()
    from concourse.bass_utils import run_bass_kernel_spmd

    q = np.asarray(q, dtype=np.float32)
    k = np.asarray(k, dtype=np.float32)
    v = np.asarray(v, dtype=np.float32)
    W1 = np.asarray(W1, dtype=np.float32)
    W2 = np.asarray(W2, dtype=np.float32)
    Wf = np.asarray(Wf, dtype=np.float32)
    bf = np.asarray(bf, dtype=np.float32)

    if "nc" not in _graph_cache:
        _graph_cache["nc"] = _build_graph()
    nc = _graph_cache["nc"]

    # q @ W2 is (B,128)@(128,128): do it on host, it is the per-batch tanh
    # bias.  bf shifts every logit of a batch equally -> softmax-invariant,
    # so it drops out of both outputs.
    c = (q[:, 0, :].astype(np.float64) @ W2.astype(np.float64)).astype(np.float32)

    k_bf = np.ascontiguousarray(k.astype(_BF16).transpose(0, 2, 1))
    v_bf = v.astype(_BF16)
    w1_bf = W1.astype(_BF16)
    wf_bf = Wf.astype(_BF16)
    ident = np.eye(D, dtype=np.float32)

    in_maps = []
    for i in range(NCORES):
        sl = slice(i * BPC, (i + 1) * BPC)
        in_maps.append({
            "k": k_bf[sl],
            "v": v_bf[sl],
            "w1": w1_bf,
            "wf": wf_bf,
            "ct": np.ascontiguousarray(c[sl].T),
            "ident": ident,
        })

    res = run_bass_kernel_spmd(
        nc, in_maps, list(range(NCORES)),
        trace=bool(int(os.environ.get("KERNEL_TRACE", "0"))),
    )
    LAST_RESULT = res

    attn = np.concatenate([r["attn"] for r in res.results], axis=0)
    attn_sm = np.concatenate([r["attn_sm"] for r in res.results], axis=0)
    return attn.astype(np.float32), attn_sm.astype(np.float32).reshape(B, SK, 1)
